# revision 19
# baseline (speedup 1.0000x reference)
"""Trainium2 Bass kernel for nn_ACTSetTransformer (8-core data-parallel).

Strategy: pure data parallel over batch B=64 -> 8 batch elements per core.
Per batch element the 4 ISAB layers + adaptive-PMA run with all activations
resident in SBUF (H^T layout [128, 2048]); the ACT loop is collapsed using
the prefix property of the seed queries; the K_max SAB/fc tail is batched
over all 168 (b, k)-set tokens; mixture logsumexp + BCE computed on device;
host only gathers shards and averages the per-core partial sums.

Heavy matmuls run as float32r (full PE rate at free-dim >= 512); attention
internals (scores/exp/AV) run in bf16. Softmax max-subtraction is skipped:
score magnitudes are < 0.02 by construction (verified vs reference), and
K-side biases are dropped because they cancel in softmax.
"""

import contextlib
import math
import os
import sys
import tempfile
import types

import numpy as np

sys.path.insert(0, "/opt/trn_rl_repo")
sys.path.insert(0, "/root/.axon_site")

import concourse.bass as bass
import concourse.bacc as bacc
import concourse.tile as tile
from concourse import mybir
from concourse.bass_utils import run_bass_kernel_spmd

F32 = mybir.dt.float32
F32R = mybir.dt.float32r
BF16 = mybir.dt.bfloat16
AF = mybir.ActivationFunctionType
ALU = mybir.AluOpType
AX = mybir.AxisListType

D = 128
HEADS = 4
DH = 32
NUM_INDS = 32
B, N, KM = 64, 2048, 6
NCORES = 8
BSH = B // NCORES          # batch per core
NTOK = 21 * BSH            # 168 stacked set-tokens per core
RSQ = 1.0 / math.sqrt(128.0)
LOG2PI = 1.8378770664093453
LN1EM10 = -23.025850929940457

_NC_CACHE = {}


def _tri(k):
    return k * (k + 1) // 2


# ---------------------------------------------------------------------------
# device program
# ---------------------------------------------------------------------------

def _build_nc():
    nc = bacc.Bacc(None, target_bir_lowering=False)
    dp = nc.declare_dram_parameter

    x_t = dp("x_t", [BSH, 2, N], F32, isOutput=False)
    xlp0 = dp("xlp0", [128, 128], F32, isOutput=False)
    xlp1 = dp("xlp1", [128, 128], F32, isOutput=False)
    wmat = dp("wmat", [11, 4, 128, 128], F32, isOutput=False)
    wbias = dp("wbias", [128, 11, 4], F32, isOutput=False)
    indT = dp("indT", [128, 4, 32], F32, isOutput=False)
    qmbd_e = dp("qmbd", [128, 128], F32, isOutput=False)
    qmpb_e = dp("qmpb", [32, 128], F32, isOutput=False)
    hsel_e = dp("hsel", [128, 128], F32, isOutput=False)
    ident_e = dp("ident", [128, 128], F32, isOutput=False)
    sabmask_e = dp("sabmask", [NTOK, NTOK], F32, isOutput=False)
    avgsel_e = dp("avgsel", [NTOK, 112], F32, isOutput=False)
    valid48_e = dp("valid48", [48, 1], F32, isOutput=False)
    inv48_e = dp("inv48", [48, 1], F32, isOutput=False)
    validf_e = dp("validf", [1, 48], F32, isOutput=False)
    neginvf_e = dp("neginvf", [1, 48], F32, isOutput=False)
    neg23f_e = dp("neg23f", [1, 48], F32, isOutput=False)
    ctarg_e = dp("ctarg", [8, 6], F32, isOutput=False)
    fcw_e = dp("fcw", [128, 8], F32, isOutput=False)
    fcb_e = dp("fcb", [8, 1], F32, isOutput=False)

    for val in (-LOG2PI, 1e-10):
        t = nc.alloc_sbuf_tensor(f"const-f32-{val}", [128, 1], F32)
        nc.gpsimd.memset(t.ap(), val)
        nc.const_aps.aps[(F32, val)] = t.ap()
    nc.all_engine_barrier()

    pout_e = dp("pout", [48, 4], F32, isOutput=True)
    bcel_e = dp("bcel", [8, 6], F32, isOutput=True)
    llp_e = dp("llp", [128, 1], F32, isOutput=True)

    with tile.TileContext(nc) as tc, contextlib.ExitStack() as stk:
        _emit(nc, tc, locals(), stk)
    nc.compile()
    return nc


def _emit(nc, tc, ext, stk):
    STAGE = int(os.environ.get("KSTAGE", "9"))
    NB = int(os.environ.get("KNB", str(BSH)))
    NL = int(os.environ.get("KNL", "4"))
    x_t, wmat, wbias, indT = ext["x_t"], ext["wmat"], ext["wbias"], ext["indT"]

    sing = stk.enter_context(tc.tile_pool(name="sing", bufs=1))
    # --- constants / weights -> SBUF -------------------------------------
    wr = {}    # f32r weights  wr[(m, j)] [128,128]
    wvb = {}   # bf16 V weights for mab0-type
    for m in range(11):
        for j in range(4):
            t = sing.tile([128, 128], F32R, tag=f"w_{m}_{j}")
            nc.gpsimd.dma_start(out=t[:], in_=wmat[m, j, :, :])
            wr[(m, j)] = t
    for m in (0, 2, 4, 6, 8):
        t = sing.tile([128, 128], BF16, tag=f"wvb_{m}")
        nc.gpsimd.dma_start(out=t[:], in_=wmat[m, 2, :, :])
        wvb[m] = t
    bia = sing.tile([128, 11, 4], F32)
    nc.sync.dma_start(out=bia[:], in_=wbias[:])
    indt_s = sing.tile([128, 4, 32], F32R)
    nc.gpsimd.dma_start(out=indt_s[:], in_=indT[:])
    qmbd_b = sing.tile([128, 128], BF16)
    nc.gpsimd.dma_start(out=qmbd_b[:], in_=ext["qmbd_e"][:])
    qmpb = sing.tile([32, 128], F32)
    nc.sync.dma_start(out=qmpb[:], in_=ext["qmpb_e"][:])
    hsel_b = sing.tile([128, 128], BF16)
    nc.gpsimd.dma_start(out=hsel_b[:], in_=ext["hsel_e"][:])
    ident = sing.tile([128, 128], F32)
    nc.sync.dma_start(out=ident[:], in_=ext["ident_e"][:])
    identb = sing.tile([128, 128], BF16)
    nc.gpsimd.dma_start(out=identb[:], in_=ext["ident_e"][:])
    mask1 = sing.tile([128, NTOK], F32)
    nc.sync.dma_start(out=mask1[:], in_=ext["sabmask_e"][0:128, :])
    mask2 = sing.tile([40, NTOK], F32)
    nc.sync.dma_start(out=mask2[:], in_=ext["sabmask_e"][128:NTOK, :])
    avgsel1 = sing.tile([128, 112], F32R)
    nc.gpsimd.dma_start(out=avgsel1[:], in_=ext["avgsel_e"][0:128, :])
    avgsel2 = sing.tile([40, 112], F32R)
    nc.gpsimd.dma_start(out=avgsel2[:], in_=ext["avgsel_e"][128:NTOK, :])
    valid48 = sing.tile([48, 1], F32)
    nc.sync.dma_start(out=valid48[:], in_=ext["valid48_e"][:])
    inv48 = sing.tile([48, 1], F32)
    nc.sync.dma_start(out=inv48[:], in_=ext["inv48_e"][:])
    validf = sing.tile([1, 48], F32)
    nc.sync.dma_start(out=validf[:], in_=ext["validf_e"][:])
    neginvf = sing.tile([1, 48], F32)
    nc.sync.dma_start(out=neginvf[:], in_=ext["neginvf_e"][:])
    neg23f = sing.tile([1, 48], F32)
    nc.sync.dma_start(out=neg23f[:], in_=ext["neg23f_e"][:])
    ctarg = sing.tile([8, 6], F32)
    nc.sync.dma_start(out=ctarg[:], in_=ext["ctarg_e"][:])
    fcw = sing.tile([128, 8], F32R)
    nc.gpsimd.dma_start(out=fcw[:], in_=ext["fcw_e"][:])
    fcb = sing.tile([8, 1], F32)
    nc.sync.dma_start(out=fcb[:], in_=ext["fcb_e"][:])

    def bcol(m, j):
        return bia[:, m, j:j + 1]

    # V-bias broadcast tiles ([32, 128], bias along free dim) for mab0/mab1
    vbb = {}
    for m in range(9):
        t = sing.tile([32, 128], F32, tag=f"vbb_{m}")
        wb = ext["wbias"][:]
        src = bass.AP(tensor=wb.tensor, offset=wb.offset + m * 4 + 2,
                      ap=[[0, 32], [44, 128]])
        nc.sync.dma_start(out=t[:], in_=src)
        vbb[m] = t

    psb = stk.enter_context(tc.tile_pool(name="psb", bufs=6, space="PSUM"))
    ps2 = stk.enter_context(tc.tile_pool(name="ps2", bufs=2, space="PSUM"))
    work = stk.enter_context(tc.tile_pool(name="work", bufs=2))
    hpool = stk.enter_context(tc.tile_pool(name="hpool", bufs=3))
    big1 = stk.enter_context(tc.tile_pool(name="big1", bufs=1))
    big2 = stk.enter_context(tc.tile_pool(name="big2", bufs=2))
    lpp = stk.enter_context(tc.tile_pool(name="lpp", bufs=1))
    smal = stk.enter_context(tc.tile_pool(name="smal", bufs=2))
    toks = stk.enter_context(tc.tile_pool(name="toks", bufs=1))

    zf32 = sing.tile([128, 128], F32, tag="zf32")
    nc.vector.memset(zf32[:], 0.0)
    k1bd = sing.tile([128, 128], F32R, tag="k1bd")
    nc.vector.tensor_copy(k1bd[:], zf32[:])
    v1bd = sing.tile([128, 128], BF16, tag="v1bd")
    nc.vector.memset(v1bd[:], 0.0)

    # --- per-ISAB-layer precompute: inducing-point queries ----------------
    qbd_l, q0pb_l = [], []
    for l in range(4):
        m0 = 2 * l
        psq = psb.tile([128, 512], F32, tag="ps")
        nc.tensor.matmul(psq[:, 0:32], wr[(m0, 0)][:], indt_s[:, l, :], start=True, stop=True)
        q0t = smal.tile([128, 32], F32, tag=f"q0t_{l}")
        nc.vector.tensor_scalar_add(q0t[:], psq[:, 0:32], bcol(m0, 0))
        qbd = sing.tile([128, 128], BF16, tag=f"qbd_{l}")
        nc.vector.memset(qbd[:], 0.0)
        for h in range(4):
            nc.vector.tensor_scalar_mul(qbd[h * 32:(h + 1) * 32, h * 32:(h + 1) * 32],
                                        q0t[h * 32:(h + 1) * 32, :], RSQ)
        pst = psb.tile([128, 512], F32, tag="ps")
        nc.tensor.transpose(pst[0:32, 0:128], q0t[:], ident[:])
        q0pb = sing.tile([32, 128], F32, tag=f"q0pb_{l}")
        nc.vector.tensor_add(q0pb[:], pst[0:32, 0:128], vbb[m0][:])
        qbd_l.append(qbd)
        q0pb_l.append(q0pb)

    tt = toks.tile([128, NTOK], F32R)  # stacked set-tokens, transposed layout
    if STAGE < 2:
        return

    # --- main per-batch-element loop --------------------------------------
    for b in range(NB):
        ht = hpool.tile([128, N], F32R, tag="ht")
        nc.gpsimd.dma_start(out=ht[0:2, :], in_=x_t[b, :, :])
        htb = work.tile([128, N], BF16, tag="htb")
        nc.gpsimd.dma_start(out=htb[0:2, :], in_=x_t[b, :, :])

        for l in range(NL):
            m0, m1 = 2 * l, 2 * l + 1
            din = 2 if l == 0 else 128
            h_in = ht
            hb_in = htb

            # ---------------- MAB0: 32 inducing queries vs N keys --------
            # K^T (no bias: cancels in softmax) -> bf16
            kb = work.tile([128, N], BF16, tag="kb")
            for s in range(4):
                psk = psb.tile([128, 512], F32, tag="ps")
                nc.tensor.matmul(psk[:], wr[(m0, 1)][0:din, :], h_in[0:din, bass.ts(s, 512)],
                                 start=True, stop=True)
                nc.vector.tensor_copy(kb[:, bass.ts(s, 512)], psk[:])
            # V natural [n, (h,d)] bf16 with ones column at 128
            vnb = work.tile([128, 16, 132], BF16, tag="vnb")
            nc.vector.memset(vnb[:, :, 128:129], 1.0)
            for g in range(4):
                psv = psb.tile([128, 512], F32, tag="ps")
                for cc in range(4):
                    c = 4 * g + cc
                    nc.tensor.matmul(psv[:, bass.ts(cc, 128)],
                                     hb_in[0:din, bass.ts(c, 128)], wvb[m0][0:din, :],
                                     start=True, stop=True)
                nc.vector.tensor_copy(
                    vnb[:, 4 * g:4 * g + 4, 0:128],
                    psv[:].rearrange("p (a b) -> p a b", a=4))
            # scores^T chunks + exp -> A^T bf16 [n, (h,q)]
            abt = work.tile([128, 16, 128], BF16, tag="abt")
            for c in range(16):
                pss = psb.tile([128, 512], F32, tag="ps")
                nc.tensor.matmul(pss[:, 0:128], kb[:, bass.ts(c, 128)], qbd_l[l][:],
                                 start=True, stop=True)
                nc.scalar.activation(abt[:, c, :], pss[:, 0:128], AF.Exp)
            # AV + denominator (ones col): cross [(h,q), (h,d)|denom]
            psx = ps2.tile([128, 132], F32, tag="psx")
            for c in range(16):
                nc.tensor.matmul(psx[:, 0:129], abt[:, c, :], vnb[:, c, 0:129],
                                 start=(c == 0), stop=(c == 15))
            rd = smal.tile([128, 1], F32, tag="rd")
            nc.vector.reciprocal(rd[:], psx[:, 128:129])
            onat = smal.tile([32, 128], F32, tag="onat")
            for h in range(4):
                sl = slice(h * 32, (h + 1) * 32)
                nc.vector.tensor_scalar_mul(onat[0:32, sl], psx[sl, sl], rd[sl, :])
            nc.vector.tensor_add(onat[:], onat[:], q0pb_l[l][:])
            # fc_o + residual -> Hm^T [128, 32] f32r
            pst = psb.tile([128, 512], F32, tag="ps")
            nc.tensor.transpose(pst[:, 0:32], onat[:], ident[0:32, 0:32])
            ot = smal.tile([128, 32], F32R, tag="ot")
            nc.vector.tensor_copy(ot[:], pst[:, 0:32])
            psw = psb.tile([128, 512], F32, tag="ps")
            nc.tensor.matmul(psw[:, 0:32], wr[(m0, 3)][:], ot[:], start=True, stop=True)
            hm = smal.tile([128, 32], F32R, tag="hm")
            rw0 = smal.tile([128, 32], F32, tag="rw0")
            nc.vector.tensor_scalar(out=rw0[:], in0=psw[:, 0:32], scalar1=bcol(m0, 3),
                                    scalar2=0.0, op0=ALU.add, op1=ALU.max)
            nc.vector.tensor_add(hm[:], ot[:].bitcast(F32), rw0[:])

            # ---------------- MAB1: N queries vs 32 keys ------------------
            # Q1^T with bias -> f32r
            q1t = big2.tile([128, N], F32R, tag="q1t")
            for s in range(4):
                psq = psb.tile([128, 512], F32, tag="ps")
                nc.tensor.matmul(psq[:], wr[(m1, 0)][0:din, :], h_in[0:din, bass.ts(s, 512)],
                                 start=True, stop=True)
                nc.vector.tensor_scalar_add(q1t[:, bass.ts(s, 512)], psq[:], bcol(m1, 0))
            # K1 block-diag (scaled by 1/sqrt(128)), f32r
            psk1 = psb.tile([128, 512], F32, tag="ps")
            nc.tensor.matmul(psk1[:, 0:32], wr[(m1, 1)][:], hm[:], start=True, stop=True)
            for h in range(4):
                sl = slice(h * 32, (h + 1) * 32)
                nc.vector.tensor_scalar_mul(k1bd[sl, sl], psk1[sl, 0:32], RSQ)
            # V1 natural + bias -> block-diag bf16
            psv1 = psb.tile([128, 512], F32, tag="ps")
            nc.tensor.matmul(psv1[0:32, 0:128], hm[:], wr[(m1, 2)][:], start=True, stop=True)
            v1n = smal.tile([32, 128], BF16, tag="v1n")
            nc.vector.tensor_add(v1n[:], psv1[0:32, 0:128], vbb[m1][:])
            for h in range(4):
                sl = slice(h * 32, (h + 1) * 32)
                nc.vector.tensor_copy(v1bd[sl, sl], v1n[0:32, sl])
            # S1^T = K1bd^T . Q1^T  [( h,j), n]; exp -> ab1 bf16
            ab1 = work.tile([128, N], BF16, tag="ab1")
            for s in range(4):
                pss = psb.tile([128, 512], F32, tag="ps")
                nc.tensor.matmul(pss[:], k1bd[:], q1t[:, bass.ts(s, 512)], start=True, stop=True)
                nc.scalar.activation(ab1[:, bass.ts(s, 512)], pss[:], AF.Exp)
            # denom expanded [(h,d), n] then reciprocal
            rden = big1.tile([128, N], F32, tag="rden")
            for s in range(4):
                psd = psb.tile([128, 512], F32, tag="ps")
                nc.tensor.matmul(psd[:], hsel_b[:], ab1[:, bass.ts(s, 512)], start=True, stop=True)
                nc.vector.reciprocal(rden[:, bass.ts(s, 512)], psd[:])
            # AV, normalize, +Q residual -> o1r f32r
            o1r = big1.tile([128, N], F32R, tag="o1r")
            for s in range(4):
                pso = psb.tile([128, 512], F32, tag="ps")
                nc.tensor.matmul(pso[:], v1bd[:], ab1[:, bass.ts(s, 512)], start=True, stop=True)
                o1 = big2.tile([128, 512], F32, tag="o1")
                nc.vector.tensor_tensor(out=o1[:], in0=pso[:], in1=rden[:, bass.ts(s, 512)],
                                        op=ALU.mult)
                nc.vector.tensor_add(o1r[:, bass.ts(s, 512)], o1[:],
                                     q1t[:, bass.ts(s, 512)].bitcast(F32))
            # fc_o + residual -> next H^T (f32r) and bf16 copy
            ht_n = hpool.tile([128, N], F32R, tag="ht")
            htb_n = work.tile([128, N], BF16, tag="htb")
            for s in range(4):
                psw1 = psb.tile([128, 512], F32, tag="ps")
                nc.tensor.matmul(psw1[:], wr[(m1, 3)][:], o1r[:, bass.ts(s, 512)],
                                 start=True, stop=True)
                rw1 = big2.tile([128, 512], F32, tag="rw1")
                nc.scalar.activation(rw1[:], psw1[:], AF.Relu, bias=bcol(m1, 3))
                nc.vector.tensor_add(ht_n[:, bass.ts(s, 512)],
                                     o1r[:, bass.ts(s, 512)].bitcast(F32), rw1[:])
                nc.vector.tensor_copy(htb_n[:, bass.ts(s, 512)],
                                      ht_n[:, bass.ts(s, 512)].bitcast(F32))
            ht, htb = ht_n, htb_n

        # ---------------- adaptive PMA (6 seed queries, shared across k) --
        if STAGE < 3:
            continue
        kb = work.tile([128, N], BF16, tag="kb")
        for s in range(4):
            psk = psb.tile([128, 512], F32, tag="ps")
            nc.tensor.matmul(psk[:], wr[(8, 1)][:], ht[:, bass.ts(s, 512)], start=True, stop=True)
            nc.vector.tensor_copy(kb[:, bass.ts(s, 512)], psk[:])
        vnb = work.tile([128, 16, 132], BF16, tag="vnb")
        nc.vector.memset(vnb[:, :, 128:129], 1.0)
        for g in range(4):
            psv = psb.tile([128, 512], F32, tag="ps")
            for cc in range(4):
                c = 4 * g + cc
                nc.tensor.matmul(psv[:, bass.ts(cc, 128)], htb[:, bass.ts(c, 128)],
                                 wvb[8][:], start=True, stop=True)
            nc.vector.tensor_copy(
                vnb[:, 4 * g:4 * g + 4, 0:128],
                psv[:].rearrange("p (a b) -> p a b", a=4))
        abt = work.tile([128, 16, 128], BF16, tag="abt")
        for c in range(16):
            pss = psb.tile([128, 512], F32, tag="ps")
            nc.tensor.matmul(pss[:, 0:128], kb[:, bass.ts(c, 128)], qmbd_b[:],
                             start=True, stop=True)
            nc.scalar.activation(abt[:, c, :], pss[:, 0:128], AF.Exp)
        psx = ps2.tile([128, 132], F32, tag="psx")
        for c in range(16):
            nc.tensor.matmul(psx[:, 0:129], abt[:, c, :], vnb[:, c, 0:129],
                             start=(c == 0), stop=(c == 15))
        rd = smal.tile([128, 1], F32, tag="rd")
        nc.vector.reciprocal(rd[:], psx[:, 128:129])
        onat = smal.tile([32, 128], F32, tag="onat")
        for h in range(4):
            sl = slice(h * 32, (h + 1) * 32)
            nc.vector.tensor_scalar_mul(onat[0:32, sl], psx[sl, sl], rd[sl, :])
        nc.vector.tensor_add(onat[:], onat[:], qmpb[:])
        pst = psb.tile([128, 512], F32, tag="ps")
        nc.tensor.transpose(pst[:, 0:32], onat[:], ident[0:32, 0:32])
        ot = smal.tile([128, 32], F32R, tag="ot")
        nc.vector.tensor_copy(ot[:], pst[:, 0:32])
        psw = psb.tile([128, 512], F32, tag="ps")
        nc.tensor.matmul(psw[:, 0:32], wr[(8, 3)][:], ot[:], start=True, stop=True)
        rw0 = smal.tile([128, 32], F32, tag="rw0")
        nc.vector.tensor_scalar(out=rw0[:], in0=psw[:, 0:32], scalar1=bcol(8, 3),
                                scalar2=0.0, op0=ALU.add, op1=ALU.max)
        zbt = smal.tile([128, 32], F32, tag="zbt")
        nc.vector.tensor_add(zbt[:], ot[:].bitcast(F32), rw0[:])
        # scatter prefix sets into the token pool
        for k in range(6):
            nc.vector.tensor_copy(tt[:, 21 * b + _tri(k): 21 * b + _tri(k) + k + 1],
                                  zbt[:, 0:k + 1])

    # ------------------ SAB x2 over the 168 stacked tokens ----------------
    if STAGE < 4:
        return
    cur = tt
    for s_i, m in enumerate((9, 10)):
        sbq = smal.tile([128, 1], F32, tag=f"sbq_{s_i}")
        nc.vector.tensor_scalar_mul(sbq[:], bia[:, m, 0:1], RSQ)
        psq = psb.tile([128, 512], F32, tag="ps")
        nc.tensor.matmul(psq[:, 0:NTOK], wr[(m, 0)][:], cur[:], start=True, stop=True)
        qs_f = work.tile([128, NTOK], F32, tag="qs_f")
        nc.vector.tensor_scalar_add(qs_f[:], psq[:, 0:NTOK], bcol(m, 0))
        qsb = work.tile([128, NTOK], BF16, tag="qsb")
        nc.vector.tensor_scalar(out=qsb[:], in0=psq[:, 0:NTOK], scalar1=RSQ,
                                scalar2=sbq[:, :], op0=ALU.mult, op1=ALU.add)
        psk = psb.tile([128, 512], F32, tag="ps")
        nc.tensor.matmul(psk[:, 0:NTOK], wr[(m, 1)][:], cur[:], start=True, stop=True)
        ksb = work.tile([128, NTOK], BF16, tag="ksb")
        nc.vector.tensor_copy(ksb[:], psk[:, 0:NTOK])
        psv = psb.tile([128, 512], F32, tag="ps")
        nc.tensor.matmul(psv[:, 0:NTOK], wr[(m, 2)][:], cur[:], start=True, stop=True)
        vst = work.tile([128, NTOK], F32, tag="vst")
        nc.vector.tensor_scalar_add(vst[:], psv[:, 0:NTOK], bcol(m, 2))
        # V natural (two partition tiles)
        pn1 = psb.tile([128, 512], F32, tag="ps")
        nc.tensor.transpose(pn1[:, 0:128], vst[:, 0:128], ident[:])
        vsn1 = work.tile([128, 128], BF16, tag="vsn1")
        nc.vector.tensor_copy(vsn1[:], pn1[:, 0:128])
        pn2 = psb.tile([128, 512], F32, tag="ps")
        nc.tensor.transpose(pn2[0:40, 0:128], vst[:, 128:NTOK], ident[:])
        vsn2 = work.tile([40, 128], BF16, tag="vsn2")
        nc.vector.tensor_copy(vsn2[:], pn2[0:40, 0:128])
        # Q natural (residual)
        qn1 = work.tile([128, 128], F32, tag="qn1")
        pq1 = psb.tile([128, 512], F32, tag="ps")
        nc.tensor.transpose(pq1[:, 0:128], qs_f[:, 0:128], ident[:])
        nc.vector.tensor_copy(qn1[:], pq1[:, 0:128])
        qn2 = work.tile([40, 128], F32, tag="qn2")
        pq2 = psb.tile([128, 512], F32, tag="ps")
        nc.tensor.transpose(pq2[0:40, 0:128], qs_f[:, 128:NTOK], ident[:])
        nc.vector.tensor_copy(qn2[:], pq2[0:40, 0:128])

        onr = []
        for (qt0, qlen, maskt, qn) in ((0, 128, mask1, qn1), (128, 40, mask2, qn2)):
            pso = ps2.tile([128, 132], F32, tag="psx")  # reused as [*,128] AV accum
            rdns = []
            emts = []
            for h in range(4):
                sl = slice(h * 32, (h + 1) * 32)
                pss = psb.tile([128, 512], F32, tag="ps")
                nc.tensor.matmul(pss[0:qlen, 0:NTOK], qsb[sl, qt0:qt0 + qlen],
                                 ksb[sl, :], start=True, stop=True,
                                 tile_position=(h * 32, 0))
                em = smal.tile([128, NTOK], BF16, tag="em")
                epss = smal.tile([128, NTOK], F32, tag="epss")
                nc.scalar.activation(epss[0:qlen, :], pss[0:qlen, 0:NTOK], AF.Exp)
                nc.vector.tensor_tensor(out=em[0:qlen, :], in0=epss[0:qlen, :],
                                        in1=maskt[0:qlen, :], op=ALU.mult)
                dn = smal.tile([128, 1], F32, tag="dn")
                nc.vector.tensor_reduce(out=dn[0:qlen, :], in_=em[0:qlen, :],
                                        axis=AX.X, op=ALU.add)
                rdn = smal.tile([128, 1], F32, tag=f"rdn_{h}")
                nc.vector.reciprocal(rdn[0:qlen, :], dn[0:qlen, :])
                rdns.append(rdn)
                # transpose Em -> [j, i] (two column blocks)
                e1p = psb.tile([128, 512], BF16, tag="ps")
                nc.tensor.transpose(e1p[:, 0:qlen], em[0:qlen, 0:128], identb[0:qlen, 0:qlen])
                e1 = smal.tile([128, 128], BF16, tag="e1")
                nc.vector.tensor_copy(e1[:, 0:qlen], e1p[:, 0:qlen])
                e2p = psb.tile([128, 512], BF16, tag="ps")
                nc.tensor.transpose(e2p[0:40, 0:qlen], em[0:qlen, 128:NTOK], identb[0:qlen, 0:qlen])
                e2 = smal.tile([40, 128], BF16, tag="e2")
                nc.vector.tensor_copy(e2[:, 0:qlen], e2p[0:40, 0:qlen])
                emts.append((e1, e2))
            for h in range(4):
                sl = slice(h * 32, (h + 1) * 32)
                e1, e2 = emts[h]
                nc.tensor.matmul(pso[0:qlen, sl], e1[:, 0:qlen], vsn1[:, sl],
                                 start=True, stop=False)
                nc.tensor.matmul(pso[0:qlen, sl], e2[:, 0:qlen], vsn2[:, sl],
                                 start=False, stop=True)
            on = work.tile([128, 128], F32, tag="on")
            for h in range(4):
                sl = slice(h * 32, (h + 1) * 32)
                nc.vector.tensor_scalar_mul(on[0:qlen, sl], pso[0:qlen, sl],
                                            rdns[h][0:qlen, :])
            nc.vector.tensor_add(on[0:qlen, :], on[0:qlen, :], qn[0:qlen, :])
            onr.append((on, qlen))
        # back to transposed layout + fc_o
        ots = work.tile([128, NTOK], F32R, tag="ots")
        po1 = psb.tile([128, 512], F32, tag="ps")
        nc.tensor.transpose(po1[:, 0:128], onr[0][0][0:128, :], ident[:])
        nc.vector.tensor_copy(ots[:, 0:128], po1[:, 0:128])
        po2 = psb.tile([128, 512], F32, tag="ps")
        nc.tensor.transpose(po2[:, 0:40], onr[1][0][0:40, :], ident[0:40, 0:40])
        nc.vector.tensor_copy(ots[:, 128:NTOK], po2[:, 0:40])
        psw = psb.tile([128, 512], F32, tag="ps")
        nc.tensor.matmul(psw[:, 0:NTOK], wr[(m, 3)][:], ots[:], start=True, stop=True)
        rw = smal.tile([128, NTOK], F32, tag="rw")
        nc.scalar.activation(rw[:], psw[:, 0:NTOK], AF.Relu, bias=bcol(m, 3))
        nxt = toks.tile([128, NTOK], F32R, tag=f"tt{s_i + 1}")
        nc.vector.tensor_add(nxt[:], ots[:].bitcast(F32), rw[:])
        cur = nxt

    # ------------------ fc head, v-means, selection -----------------------
    if STAGE < 5:
        return
    psf = psb.tile([128, 512], F32, tag="ps")
    nc.tensor.matmul(psf[0:6, 0:NTOK], fcw[:, 0:6], cur[:], start=True, stop=True)
    fct = work.tile([6, NTOK], F32, tag="fct")
    nc.vector.tensor_scalar_add(fct[:], psf[0:6, 0:NTOK], fcb[0:6, :])
    pf1 = psb.tile([128, 512], F32, tag="ps")
    nc.tensor.transpose(pf1[:, 0:6], fct[:, 0:128], ident[0:6, 0:6])
    fn1 = smal.tile([128, 6], F32R, tag="fn1")
    nc.vector.tensor_copy(fn1[:], pf1[:, 0:6])
    pf2 = psb.tile([128, 512], F32, tag="ps")
    nc.tensor.transpose(pf2[0:40, 0:6], fct[:, 128:NTOK], ident[0:6, 0:6])
    fn2 = smal.tile([40, 6], F32R, tag="fn2")
    nc.vector.tensor_copy(fn2[:], pf2[0:40, 0:6])
    psvs = psb.tile([128, 512], F32, tag="ps")
    nc.tensor.matmul(psvs[0:112, 0:6], avgsel1[:], fn1[:], start=True, stop=False)
    nc.tensor.matmul(psvs[0:112, 0:6], avgsel2[:], fn2[:], start=False, stop=True)
    sel = work.tile([48, 6], F32, tag="sel")
    nc.vector.tensor_copy(sel[0:32, :], psvs[64:96, 0:6])
    nc.vector.tensor_copy(sel[32:48, :], psvs[96:112, 0:6])
    vl48 = smal.tile([48, 1], F32, tag="vl48")
    nc.vector.tensor_copy(vl48[:], psvs[0:48, 0:1])

    # ------------------ params_out ---------------------------------------
    pout = work.tile([48, 4], F32, tag="pout")
    nc.vector.tensor_copy(pout[:, 0:2], sel[:, 2:4])
    nc.scalar.activation(pout[:, 2:4], sel[:, 4:6], AF.Exp)
    nc.scalar.activation(pout[:, 2:4], pout[:, 2:4], AF.Ln, bias=1.0)
    nc.vector.tensor_scalar_mul(pout[:], pout[:], valid48[:, :])
    nc.vector.tensor_scalar_add(pout[:], pout[:], inv48[:, :])
    nc.sync.dma_start(out=ext["pout_e"][:], in_=pout[:])

    if STAGE < 6:
        return
    # ------------------ mixture weights (pi) ------------------------------
    pst = psb.tile([128, 512], F32, tag="ps")
    nc.tensor.transpose(pst[0:1, 0:48], sel[:, 1:2], ident[0:48, 0:48])
    pl = smal.tile([1, 48], F32, tag="pl")
    nc.vector.tensor_tensor(out=pl[:], in0=pst[0:1, 0:48], in1=validf[:], op=ALU.mult)
    nc.vector.tensor_add(pl[:], pl[:], neginvf[:])
    epl = smal.tile([1, 48], F32, tag="epl")
    nc.scalar.activation(epl[:], pl[:], AF.Exp)
    den8 = smal.tile([1, 8], F32, tag="den8")
    nc.vector.tensor_reduce(out=den8[:], in_=epl[:].rearrange("p (a b) -> p a b", a=8),
                            axis=AX.X, op=ALU.add)
    ld8 = smal.tile([1, 8], F32, tag="ld8")
    nc.scalar.activation(ld8[:], den8[:], AF.Ln)
    ldb = smal.tile([1, 48], F32, tag="ldb")
    ld8_ap = ld8[:]
    nc.vector.tensor_copy(ldb[:].rearrange("p (a b) -> p a b", a=8),
                          bass.AP(tensor=ld8_ap.tensor, offset=ld8_ap.offset,
                                  ap=[ld8_ap.ap[0], [1, 8], [0, 6]]))
    lpi = smal.tile([1, 48], F32, tag="lpi")
    nc.vector.tensor_sub(lpi[:], pl[:], ldb[:])
    nc.vector.tensor_tensor(out=lpi[:], in0=lpi[:], in1=validf[:], op=ALU.mult)
    nc.vector.tensor_add(lpi[:], lpi[:], neg23f[:])

    # sigma stats for the log-prob stage
    lg = smal.tile([48, 2], F32, tag="lg")
    nc.scalar.activation(lg[:], pout[:, 2:4], AF.Ln)
    lgs = smal.tile([48, 1], F32, tag="lgs")
    nc.vector.tensor_reduce(out=lgs[:], in_=lg[:], axis=AX.X, op=ALU.add)
    rs = smal.tile([48, 2], F32, tag="rs")
    nc.vector.reciprocal(rs[:], pout[:, 2:4])

    # stage to DRAM: pk[48, 8] = mu0 mu1 rs0 rs1 lgs lpi
    dram = stk.enter_context(tc.tile_pool(name="dram", bufs=1, space="DRAM"))
    pk = dram.tile([48, 8], F32)
    nc.sync.dma_start(out=pk[:, 0:2], in_=pout[:, 0:2])
    nc.sync.dma_start(out=pk[:, 2:4], in_=rs[:])
    nc.sync.dma_start(out=pk[:, 4:5], in_=lgs[:])
    pk_ap = pk[:]
    nc.sync.dma_start(out=bass.AP(tensor=pk_ap.tensor, offset=pk_ap.offset + 5, ap=[[8, 48]]),
                      in_=lpi[:])
    vstg = dram.tile([48, 1], F32)
    nc.sync.dma_start(out=vstg[:], in_=vl48[:])

    # ------------------ log-likelihood over all (b, n) --------------------
    def pk_bcast(col):
        t = smal.tile([128, 6], F32, tag=f"pkb_{col}")
        pk_ap = pk[:]
        for k in range(6):
            src = bass.AP(tensor=pk_ap.tensor, offset=pk_ap.offset + k * 8 + col,
                          ap=[[48, 8], [0, 16]])
            nc.sync.dma_start(out=t[:, k:k + 1], in_=src)
        return t

    if STAGE < 7:
        return
    mu0b, mu1b = pk_bcast(0), pk_bcast(1)
    rs0b, rs1b = pk_bcast(2), pk_bcast(3)
    lgsb, lpib = pk_bcast(4), pk_bcast(5)
    ec = smal.tile([128, 6], F32, tag="ec")
    nc.vector.tensor_sub(ec[:], lpib[:], lgsb[:])
    nc.scalar.activation(ec[:], ec[:], AF.Exp, bias=-LOG2PI)
    xs0 = work.tile([128, 128], F32, tag="xs0")
    nc.sync.dma_start(out=xs0[:], in_=ext["xlp0"][:])
    xs1 = work.tile([128, 128], F32, tag="xs1")
    nc.sync.dma_start(out=xs1[:], in_=ext["xlp1"][:])

    def bc6(t):  # [128, 6] -> broadcast over nl (inner layout nl,k)
        a = t[:]
        return bass.AP(tensor=a.tensor, offset=a.offset, ap=[a.ap[0], [0, 128], [1, 6]])

    def bcx(t):  # [128, 128] x -> repeat over k
        a = t[:]
        return bass.AP(tensor=a.tensor, offset=a.offset, ap=[a.ap[0], [1, 128], [0, 6]])

    acc = lpp.tile([128, 128, 6], F32, tag="acc")
    z1t = lpp.tile([128, 128, 6], F32, tag="z1t")
    nc.vector.tensor_tensor(out=acc[:], in0=bcx(xs0), in1=bc6(mu0b), op=ALU.subtract)
    nc.vector.tensor_tensor(out=acc[:], in0=acc[:], in1=bc6(rs0b), op=ALU.mult)
    nc.scalar.activation(acc[:], acc[:], AF.Square)
    nc.vector.tensor_tensor(out=z1t[:], in0=bcx(xs1), in1=bc6(mu1b), op=ALU.subtract)
    nc.vector.tensor_tensor(out=z1t[:], in0=z1t[:], in1=bc6(rs1b), op=ALU.mult)
    nc.scalar.activation(z1t[:], z1t[:], AF.Square)
    nc.vector.tensor_add(acc[:], acc[:], z1t[:])
    nc.scalar.activation(acc[:], acc[:], AF.Exp, scale=-0.5)
    nc.vector.tensor_tensor(out=acc[:], in0=acc[:], in1=bc6(ec), op=ALU.mult)
    ssum = work.tile([128, 128], F32, tag="ssum")
    nc.vector.tensor_reduce(out=ssum[:], in_=acc[:], axis=AX.X, op=ALU.add)
    nc.scalar.activation(ssum[:], ssum[:], AF.Ln)
    llp = smal.tile([128, 1], F32, tag="llp")
    nc.vector.tensor_reduce(out=llp[:], in_=ssum[:], axis=AX.X, op=ALU.add)
    nc.sync.dma_start(out=ext["llp_e"][:], in_=llp[:])

    if STAGE < 8:
        return
    # ------------------ BCE on cumulative halting logits -------------------
    vl8 = smal.tile([8, 6], F32, tag="vl8")
    vstg_ap = vstg[:]
    nc.sync.dma_start(out=vl8[:], in_=bass.AP(tensor=vstg_ap.tensor, offset=vstg_ap.offset,
                                              ap=[[6, 8], [1, 6]]))
    sg = smal.tile([8, 6], F32, tag="sg")
    nc.scalar.activation(sg[:], vl8[:], AF.Sigmoid)
    lv = smal.tile([8, 6], F32, tag="lv")
    nc.scalar.activation(lv[:], sg[:], AF.Ln, bias=1e-10)
    lc = smal.tile([8, 6], F32, tag="lc")
    nc.vector.tensor_copy(lc[:], lv[:])
    for j in range(1, 6):
        nc.vector.tensor_add(lc[:, j:j + 1], lc[:, j - 1:j], lv[:, j:j + 1])
    exl = smal.tile([8, 6], F32, tag="exl")
    nc.scalar.activation(exl[:], lc[:], AF.Exp)
    t1 = smal.tile([8, 6], F32, tag="t1")
    nc.vector.tensor_scalar(out=t1[:], in0=exl[:], scalar1=-1.0, scalar2=1.0 + 1e-10,
                            op0=ALU.mult, op1=ALU.add)
    nc.scalar.activation(t1[:], t1[:], AF.Ln)
    cl = smal.tile([8, 6], F32, tag="cl")
    nc.vector.tensor_sub(cl[:], lc[:], t1[:])
    mx = smal.tile([8, 6], F32, tag="mx")
    nc.vector.tensor_scalar_max(mx[:], cl[:], 0.0)
    ct = smal.tile([8, 6], F32, tag="ct")
    nc.vector.tensor_tensor(out=ct[:], in0=cl[:], in1=ctarg[:], op=ALU.mult)
    nc.vector.tensor_sub(mx[:], mx[:], ct[:])
    ab = smal.tile([8, 6], F32, tag="ab")
    nc.scalar.activation(ab[:], cl[:], AF.Abs)
    nc.scalar.activation(ab[:], ab[:], AF.Exp, scale=-1.0)
    nc.scalar.activation(ab[:], ab[:], AF.Ln, bias=1.0)
    bcel = smal.tile([8, 6], F32, tag="bcel")
    nc.vector.tensor_add(bcel[:], mx[:], ab[:])
    nc.sync.dma_start(out=ext["bcel_e"][:], in_=bcel[:])


# ---------------------------------------------------------------------------
# host side
# ---------------------------------------------------------------------------

def _np(x):
    return np.asarray(x, dtype=np.float32)


def _prep_inputs(X, params, K_true):
    """Build per-core input maps."""
    wmat = np.zeros((11, 4, 128, 128), np.float32)
    wbias = np.zeros((128, 11, 4), np.float32)
    indT = np.zeros((128, 4, 32), np.float32)
    mabs = []
    for l in range(4):
        mabs.append(params["isab"][l]["mab0"])
        mabs.append(params["isab"][l]["mab1"])
    mabs.append(params["apma"]["mab"])
    mabs.append(params["sab"][0])
    mabs.append(params["sab"][1])
    for m, p in enumerate(mabs):
        for j, key in enumerate(("q", "k", "v", "o")):
            w = _np(p[key]["w"])
            wmat[m, j, :w.shape[0], :] = w
            wbias[:, m, j] = _np(p[key]["b"])
    for l in range(4):
        indT[:, l, :] = _np(params["isab"][l]["I"])[0].T

    # seed recurrence (parameter-only preprocessing)
    sfw = _np(params["apma"]["seed_fc"]["w"])
    sfb = _np(params["apma"]["seed_fc"]["b"])
    s = _np(params["apma"]["seed"])
    seeds = np.zeros((6, 128), np.float32)
    for k in range(6):
        seeds[k] = s
        s = np.tanh(s @ sfw + sfb)
    qm = seeds @ _np(params["apma"]["mab"]["q"]["w"]) + _np(params["apma"]["mab"]["q"]["b"])
    qmbd = np.zeros((128, 128), np.float32)
    for h in range(4):
        qmbd[h * 32:(h + 1) * 32, h * 32:h * 32 + 6] = qm.T[h * 32:(h + 1) * 32] * RSQ
    qmpb = np.zeros((32, 128), np.float32)
    qmpb[0:6] = qm + _np(params["apma"]["mab"]["v"]["b"])[None, :]

    hsel = np.zeros((128, 128), np.float32)
    for h in range(4):
        hsel[h * 32:(h + 1) * 32, h * 32:(h + 1) * 32] = 1.0
    ident = np.eye(128, dtype=np.float32)

    # token bookkeeping: token t = b*21 + tri(k) + i
    sabmask = np.zeros((NTOK, NTOK), np.float32)
    setid = np.empty(NTOK, np.int64)
    for b in range(BSH):
        for k in range(6):
            t0 = 21 * b + _tri(k)
            setid[t0:t0 + k + 1] = b * 6 + k
    sabmask[setid[:, None] == setid[None, :]] = 1.0

    avg = np.zeros((NTOK, 112), np.float32)
    for b in range(BSH):
        for k in range(6):
            t0 = 21 * b + _tri(k)
            avg[t0:t0 + k + 1, b * 6 + k] = 1.0 / (k + 1)

    fcw = np.zeros((128, 8), np.float32)
    fcw[:, 0:6] = _np(params["fc"]["w"])
    fcb = np.zeros((8, 1), np.float32)
    fcb[0:6, 0] = _np(params["fc"]["b"])

    X = _np(X)
    K_true = np.asarray(K_true).astype(np.int64)
    in_maps = []
    for c in range(NCORES):
        Xc = X[c * BSH:(c + 1) * BSH]           # [8, 2048, 2]
        kt = K_true[c * BSH:(c + 1) * BSH]      # [8]
        avgsel = avg.copy()
        for b in range(BSH):
            kb = int(kt[b])
            t0 = 21 * b + _tri(kb - 1)
            for j in range(kb):
                avgsel[t0 + j, 64 + b * 6 + j] = 1.0
        valid = (np.arange(6)[None, :] < kt[:, None]).astype(np.float32)  # [8, 6]
        v48 = valid.reshape(48, 1)
        ctarg = (np.arange(6)[None, :] < (kt - 1)[:, None]).astype(np.float32)
        in_maps.append({
            "x_t": np.ascontiguousarray(Xc.transpose(0, 2, 1)),
            "xlp0": np.ascontiguousarray(Xc[:, :, 0].reshape(128, 128)),
            "xlp1": np.ascontiguousarray(Xc[:, :, 1].reshape(128, 128)),
            "wmat": wmat, "wbias": wbias, "indT": indT,
            "qmbd": qmbd, "qmpb": qmpb, "hsel": hsel, "ident": ident,
            "sabmask": sabmask, "avgsel": avgsel,
            "valid48": v48, "inv48": 1.0 - v48,
            "validf": v48.reshape(1, 48).copy(),
            "neginvf": ((v48 - 1.0) * 1e10).reshape(1, 48).copy(),
            "neg23f": ((1.0 - v48) * LN1EM10).reshape(1, 48).copy(),
            "ctarg": ctarg, "fcw": fcw, "fcb": fcb,
        })
    return in_maps


def kernel(X, params, K_true, K_max):
    assert int(K_max) == KM
    if "nc" not in _NC_CACHE:
        _NC_CACHE["nc"] = _build_nc()
    nc = _NC_CACHE["nc"]
    in_maps = _prep_inputs(X, params, K_true)
    res = run_bass_kernel_spmd(nc, in_maps, core_ids=list(range(NCORES)))
    pouts, lls, bcs = [], [], []
    for c in range(NCORES):
        r = res.results[c]
        pouts.append(r["pout"].reshape(BSH, 6, 4))
        lls.append(r["llp"].sum())
        bcs.append(r["bcel"].mean())
    params_out = np.concatenate(pouts, 0).astype(np.float32)
    ll = np.float32(np.sum(lls) / (B * N))
    bc = np.float32(np.mean(bcs))
    return params_out, ll, bc


# revision 21
# speedup vs baseline: 1.2323x; 1.2323x over previous
"""Trainium2 Bass kernel for nn_ACTSetTransformer (8-core data-parallel).

Strategy: pure data parallel over batch B=64 -> 8 batch elements per core.
Per batch element the 4 ISAB layers + adaptive-PMA run with all activations
resident in SBUF (H^T layout [128, 2048]); the ACT loop is collapsed using
the prefix property of the seed queries; the K_max SAB/fc tail is batched
over all 168 (b, k)-set tokens; mixture logsumexp + BCE computed on device;
host only gathers shards and averages the per-core partial sums.

Heavy matmuls run as float32r (full PE rate at free-dim >= 512); attention
internals (scores/exp/AV) run in bf16. Softmax max-subtraction is skipped:
score magnitudes are < 0.02 by construction (verified vs reference), and
K-side biases are dropped because they cancel in softmax.
"""

import contextlib
import math
import os
import sys
import tempfile
import types

import numpy as np

sys.path.insert(0, "/opt/trn_rl_repo")
sys.path.insert(0, "/root/.axon_site")

import concourse.bass as bass
import concourse.bacc as bacc
import concourse.tile as tile
from concourse import mybir
from concourse.bass_utils import run_bass_kernel_spmd

F32 = mybir.dt.float32
F32R = mybir.dt.float32r
BF16 = mybir.dt.bfloat16
AF = mybir.ActivationFunctionType
ALU = mybir.AluOpType
AX = mybir.AxisListType

D = 128
HEADS = 4
DH = 32
NUM_INDS = 32
B, N, KM = 64, 2048, 6
NCORES = 8
BSH = B // NCORES          # batch per core
NTOK = 21 * BSH            # 168 stacked set-tokens per core
RSQ = 1.0 / math.sqrt(128.0)
LOG2PI = 1.8378770664093453
LN1EM10 = -23.025850929940457

_NC_CACHE = {}


def _tri(k):
    return k * (k + 1) // 2


# ---------------------------------------------------------------------------
# device program
# ---------------------------------------------------------------------------

def _build_nc():
    nc = bacc.Bacc(None, target_bir_lowering=False)
    dp = nc.declare_dram_parameter

    x_t = dp("x_t", [BSH, 2, N], F32, isOutput=False)
    xlp0 = dp("xlp0", [128, 128], F32, isOutput=False)
    xlp1 = dp("xlp1", [128, 128], F32, isOutput=False)
    wmat = dp("wmat", [11, 4, 128, 128], F32, isOutput=False)
    wbias = dp("wbias", [128, 11, 4], F32, isOutput=False)
    indT = dp("indT", [128, 4, 32], F32, isOutput=False)
    qmbd_e = dp("qmbd", [128, 128], F32, isOutput=False)
    qmpb_e = dp("qmpb", [32, 128], F32, isOutput=False)
    hsel_e = dp("hsel", [128, 128], F32, isOutput=False)
    ident_e = dp("ident", [128, 128], F32, isOutput=False)
    sabmask_e = dp("sabmask", [NTOK, NTOK], F32, isOutput=False)
    avgsel_e = dp("avgsel", [NTOK, 112], F32, isOutput=False)
    valid48_e = dp("valid48", [48, 1], F32, isOutput=False)
    inv48_e = dp("inv48", [48, 1], F32, isOutput=False)
    validf_e = dp("validf", [1, 48], F32, isOutput=False)
    neginvf_e = dp("neginvf", [1, 48], F32, isOutput=False)
    neg23f_e = dp("neg23f", [1, 48], F32, isOutput=False)
    ctarg_e = dp("ctarg", [8, 6], F32, isOutput=False)
    fcw_e = dp("fcw", [128, 8], F32, isOutput=False)
    fcb_e = dp("fcb", [8, 1], F32, isOutput=False)

    for val in (-LOG2PI, 1e-10):
        t = nc.alloc_sbuf_tensor(f"const-f32-{val}", [128, 1], F32)
        nc.gpsimd.memset(t.ap(), val)
        nc.const_aps.aps[(F32, val)] = t.ap()
    nc.all_engine_barrier()

    pout_e = dp("pout", [48, 4], F32, isOutput=True)
    bcel_e = dp("bcel", [8, 6], F32, isOutput=True)
    llp_e = dp("llp", [128, 1], F32, isOutput=True)

    with tile.TileContext(nc) as tc, contextlib.ExitStack() as stk:
        _emit(nc, tc, locals(), stk)
    nc.compile()
    return nc


def _emit(nc, tc, ext, stk):
    STAGE = int(os.environ.get("KSTAGE", "9"))
    NB = int(os.environ.get("KNB", str(BSH)))
    NL = int(os.environ.get("KNL", "4"))
    x_t, wmat, wbias, indT = ext["x_t"], ext["wmat"], ext["wbias"], ext["indT"]

    sing = stk.enter_context(tc.tile_pool(name="sing", bufs=1))
    # --- constants / weights -> SBUF -------------------------------------
    wr = {}    # f32r weights  wr[(m, j)] [128,128]
    wvb = {}   # bf16 V weights for mab0-type
    for m in range(11):
        for j in range(4):
            t = sing.tile([128, 128], F32R, tag=f"w_{m}_{j}")
            nc.gpsimd.dma_start(out=t[:], in_=wmat[m, j, :, :])
            wr[(m, j)] = t
    for m in (0, 2, 4, 6, 8):
        t = sing.tile([128, 128], BF16, tag=f"wvb_{m}")
        nc.gpsimd.dma_start(out=t[:], in_=wmat[m, 2, :, :])
        wvb[m] = t
    bia = sing.tile([128, 11, 4], F32)
    nc.sync.dma_start(out=bia[:], in_=wbias[:])
    indt_s = sing.tile([128, 4, 32], F32R)
    nc.gpsimd.dma_start(out=indt_s[:], in_=indT[:])
    qmbd_b = sing.tile([128, 128], BF16)
    nc.gpsimd.dma_start(out=qmbd_b[:], in_=ext["qmbd_e"][:])
    qmpb = sing.tile([32, 128], F32)
    nc.sync.dma_start(out=qmpb[:], in_=ext["qmpb_e"][:])
    hsel_b = sing.tile([128, 128], BF16)
    nc.gpsimd.dma_start(out=hsel_b[:], in_=ext["hsel_e"][:])
    ident = sing.tile([128, 128], F32)
    nc.sync.dma_start(out=ident[:], in_=ext["ident_e"][:])
    identb = sing.tile([128, 128], BF16)
    nc.gpsimd.dma_start(out=identb[:], in_=ext["ident_e"][:])
    mask1 = sing.tile([128, NTOK], F32)
    nc.sync.dma_start(out=mask1[:], in_=ext["sabmask_e"][0:128, :])
    mask2 = sing.tile([40, NTOK], F32)
    nc.sync.dma_start(out=mask2[:], in_=ext["sabmask_e"][128:NTOK, :])
    avgsel1 = sing.tile([128, 112], F32R)
    nc.gpsimd.dma_start(out=avgsel1[:], in_=ext["avgsel_e"][0:128, :])
    avgsel2 = sing.tile([40, 112], F32R)
    nc.gpsimd.dma_start(out=avgsel2[:], in_=ext["avgsel_e"][128:NTOK, :])
    valid48 = sing.tile([48, 1], F32)
    nc.sync.dma_start(out=valid48[:], in_=ext["valid48_e"][:])
    inv48 = sing.tile([48, 1], F32)
    nc.sync.dma_start(out=inv48[:], in_=ext["inv48_e"][:])
    validf = sing.tile([1, 48], F32)
    nc.sync.dma_start(out=validf[:], in_=ext["validf_e"][:])
    neginvf = sing.tile([1, 48], F32)
    nc.sync.dma_start(out=neginvf[:], in_=ext["neginvf_e"][:])
    neg23f = sing.tile([1, 48], F32)
    nc.sync.dma_start(out=neg23f[:], in_=ext["neg23f_e"][:])
    ctarg = sing.tile([8, 6], F32)
    nc.sync.dma_start(out=ctarg[:], in_=ext["ctarg_e"][:])
    fcw = sing.tile([128, 8], F32R)
    nc.gpsimd.dma_start(out=fcw[:], in_=ext["fcw_e"][:])
    fcb = sing.tile([8, 1], F32)
    nc.sync.dma_start(out=fcb[:], in_=ext["fcb_e"][:])

    def bcol(m, j):
        return bia[:, m, j:j + 1]

    # V-bias broadcast tiles ([32, 128], bias along free dim) for mab0/mab1
    vbb = {}
    for m in range(9):
        t = sing.tile([32, 128], F32, tag=f"vbb_{m}")
        wb = ext["wbias"][:]
        src = bass.AP(tensor=wb.tensor, offset=wb.offset + m * 4 + 2,
                      ap=[[0, 32], [44, 128]])
        nc.sync.dma_start(out=t[:], in_=src)
        vbb[m] = t

    psb = stk.enter_context(tc.tile_pool(name="psb", bufs=6, space="PSUM"))
    ps2 = stk.enter_context(tc.tile_pool(name="ps2", bufs=2, space="PSUM"))
    work = stk.enter_context(tc.tile_pool(name="work", bufs=2))
    hpool = stk.enter_context(tc.tile_pool(name="hpool", bufs=3))
    big1 = stk.enter_context(tc.tile_pool(name="big1", bufs=1))
    big2 = stk.enter_context(tc.tile_pool(name="big2", bufs=2))
    lpp = stk.enter_context(tc.tile_pool(name="lpp", bufs=1))
    smal = stk.enter_context(tc.tile_pool(name="smal", bufs=2))
    toks = stk.enter_context(tc.tile_pool(name="toks", bufs=1))

    zf32 = sing.tile([128, 128], F32, tag="zf32")
    nc.vector.memset(zf32[:], 0.0)
    k1bd = sing.tile([128, 128], F32R, tag="k1bd")
    nc.vector.tensor_copy(k1bd[:], zf32[:])
    v1bd = sing.tile([128, 128], BF16, tag="v1bd")
    nc.vector.memset(v1bd[:], 0.0)

    # --- per-ISAB-layer precompute: inducing-point queries ----------------
    qbd_l, q0pb_l = [], []
    for l in range(4):
        m0 = 2 * l
        psq = psb.tile([128, 512], F32, tag="ps")
        nc.tensor.matmul(psq[:, 0:32], wr[(m0, 0)][:], indt_s[:, l, :], start=True, stop=True)
        q0t = smal.tile([128, 32], F32, tag=f"q0t_{l}")
        nc.vector.tensor_scalar_add(q0t[:], psq[:, 0:32], bcol(m0, 0))
        qbd = sing.tile([128, 128], BF16, tag=f"qbd_{l}")
        nc.vector.memset(qbd[:], 0.0)
        for h in range(4):
            nc.vector.tensor_scalar_mul(qbd[h * 32:(h + 1) * 32, h * 32:(h + 1) * 32],
                                        q0t[h * 32:(h + 1) * 32, :], RSQ)
        pst = psb.tile([128, 512], F32, tag="ps")
        nc.tensor.transpose(pst[0:32, 0:128], q0t[:], ident[:])
        q0pb = sing.tile([32, 128], F32, tag=f"q0pb_{l}")
        nc.vector.tensor_add(q0pb[:], pst[0:32, 0:128], vbb[m0][:])
        qbd_l.append(qbd)
        q0pb_l.append(q0pb)

    tt = toks.tile([128, NTOK], F32R)  # stacked set-tokens, transposed layout
    if STAGE < 2:
        return

    # --- main per-batch-element loop --------------------------------------
    for b in range(NB):
        ht = hpool.tile([128, N], F32R, tag="ht")
        nc.gpsimd.dma_start(out=ht[0:2, :], in_=x_t[b, :, :])
        htb = work.tile([128, N], BF16, tag="htb")
        nc.gpsimd.dma_start(out=htb[0:2, :], in_=x_t[b, :, :])

        for l in range(NL):
            m0, m1 = 2 * l, 2 * l + 1
            din = 2 if l == 0 else 128
            h_in = ht
            hb_in = htb

            # ---------------- MAB0: 32 inducing queries vs N keys --------
            # K^T (no bias: cancels in softmax) -> bf16
            kb = work.tile([128, N], BF16, tag="kb")
            for s in range(4):
                psk = psb.tile([128, 512], F32, tag="ps")
                nc.tensor.matmul(psk[:], wr[(m0, 1)][0:din, :], h_in[0:din, bass.ts(s, 512)],
                                 start=True, stop=True)
                nc.scalar.activation(kb[:, bass.ts(s, 512)], psk[:], AF.Copy)
            # V natural [n, (h,d)] bf16 with ones column at 128
            vnb = work.tile([128, 16, 132], BF16, tag="vnb")
            nc.vector.memset(vnb[:, :, 128:129], 1.0)
            for g in range(4):
                psv = psb.tile([128, 512], F32, tag="ps")
                for cc in range(4):
                    c = 4 * g + cc
                    nc.tensor.matmul(psv[:, bass.ts(cc, 128)],
                                     hb_in[0:din, bass.ts(c, 128)], wvb[m0][0:din, :],
                                     start=True, stop=True)
                nc.vector.tensor_copy(
                    vnb[:, 4 * g:4 * g + 4, 0:128],
                    psv[:].rearrange("p (a b) -> p a b", a=4))
            # scores^T chunks + exp -> A^T bf16 [n, (h,q)]
            abt = work.tile([128, 16, 128], BF16, tag="abt")
            for c in range(16):
                pss = psb.tile([128, 512], F32, tag="ps")
                nc.tensor.matmul(pss[:, 0:128], kb[:, bass.ts(c, 128)], qbd_l[l][:],
                                 start=True, stop=True)
                nc.scalar.activation(abt[:, c, :], pss[:, 0:128], AF.Exp)
            # AV + denominator (ones col): cross [(h,q), (h,d)|denom]
            psx = ps2.tile([128, 132], F32, tag="psx")
            for c in range(16):
                nc.tensor.matmul(psx[:, 0:129], abt[:, c, :], vnb[:, c, 0:129],
                                 start=(c == 0), stop=(c == 15))
            rd = smal.tile([128, 1], F32, tag="rd")
            nc.vector.reciprocal_approx_fast(rd[:], psx[:, 128:129])
            onat = smal.tile([32, 128], F32, tag="onat")
            for h in range(4):
                sl = slice(h * 32, (h + 1) * 32)
                nc.vector.tensor_scalar_mul(onat[0:32, sl], psx[sl, sl], rd[sl, :])
            nc.vector.tensor_add(onat[:], onat[:], q0pb_l[l][:])
            # fc_o + residual -> Hm^T [128, 32] f32r
            pst = psb.tile([128, 512], F32, tag="ps")
            nc.tensor.transpose(pst[:, 0:32], onat[:], ident[0:32, 0:32])
            ot = smal.tile([128, 32], F32R, tag="ot")
            nc.vector.tensor_copy(ot[:], pst[:, 0:32])
            psw = psb.tile([128, 512], F32, tag="ps")
            nc.tensor.matmul(psw[:, 0:32], wr[(m0, 3)][:], ot[:], start=True, stop=True)
            hm = smal.tile([128, 32], F32R, tag="hm")
            rw0 = smal.tile([128, 32], F32, tag="rw0")
            nc.vector.tensor_scalar(out=rw0[:], in0=psw[:, 0:32], scalar1=bcol(m0, 3),
                                    scalar2=0.0, op0=ALU.add, op1=ALU.max)
            nc.vector.tensor_add(hm[:], ot[:].bitcast(F32), rw0[:])

            # ---------------- MAB1: N queries vs 32 keys ------------------
            # Q1^T with bias -> f32r
            q1t = big2.tile([128, N], F32R, tag="q1t")
            for s in range(4):
                psq = psb.tile([128, 512], F32, tag="ps")
                nc.tensor.matmul(psq[:], wr[(m1, 0)][0:din, :], h_in[0:din, bass.ts(s, 512)],
                                 start=True, stop=True)
                nc.scalar.activation(q1t[:, bass.ts(s, 512)], psq[:], AF.Identity, bias=bcol(m1, 0))
            # K1 block-diag (scaled by 1/sqrt(128)), f32r
            psk1 = psb.tile([128, 512], F32, tag="ps")
            nc.tensor.matmul(psk1[:, 0:32], wr[(m1, 1)][:], hm[:], start=True, stop=True)
            for h in range(4):
                sl = slice(h * 32, (h + 1) * 32)
                nc.vector.tensor_scalar_mul(k1bd[sl, sl], psk1[sl, 0:32], RSQ)
            # V1 natural + bias -> block-diag bf16
            psv1 = psb.tile([128, 512], F32, tag="ps")
            nc.tensor.matmul(psv1[0:32, 0:128], hm[:], wr[(m1, 2)][:], start=True, stop=True)
            v1n = smal.tile([32, 128], BF16, tag="v1n")
            nc.vector.tensor_add(v1n[:], psv1[0:32, 0:128], vbb[m1][:])
            for h in range(4):
                sl = slice(h * 32, (h + 1) * 32)
                nc.vector.tensor_copy(v1bd[sl, sl], v1n[0:32, sl])
            # S1^T = K1bd^T . Q1^T  [( h,j), n]; exp -> ab1 bf16
            ab1 = work.tile([128, N], BF16, tag="ab1")
            for s in range(4):
                pss = psb.tile([128, 512], F32, tag="ps")
                nc.tensor.matmul(pss[:], k1bd[:], q1t[:, bass.ts(s, 512)], start=True, stop=True)
                nc.scalar.activation(ab1[:, bass.ts(s, 512)], pss[:], AF.Exp)
            # denom expanded [(h,d), n] then reciprocal
            rden = big1.tile([128, N], F32, tag="rden")
            for s in range(4):
                psd = psb.tile([128, 512], F32, tag="ps")
                nc.tensor.matmul(psd[:], hsel_b[:], ab1[:, bass.ts(s, 512)], start=True, stop=True)
                nc.vector.reciprocal_approx_fast(rden[:, bass.ts(s, 512)], psd[:])
            # AV, normalize, +Q residual -> o1r f32r
            o1r = big1.tile([128, N], F32R, tag="o1r")
            for s in range(4):
                pso = psb.tile([128, 512], F32, tag="ps")
                nc.tensor.matmul(pso[:], v1bd[:], ab1[:, bass.ts(s, 512)], start=True, stop=True)
                o1 = big2.tile([128, 512], F32, tag="o1")
                nc.vector.tensor_tensor(out=o1[:], in0=pso[:], in1=rden[:, bass.ts(s, 512)],
                                        op=ALU.mult)
                nc.vector.tensor_add(o1r[:, bass.ts(s, 512)], o1[:],
                                     q1t[:, bass.ts(s, 512)].bitcast(F32))
            # fc_o + residual -> next H^T (f32r) and bf16 copy
            ht_n = hpool.tile([128, N], F32R, tag="ht")
            htb_n = work.tile([128, N], BF16, tag="htb")
            for s in range(4):
                psw1 = psb.tile([128, 512], F32, tag="ps")
                nc.tensor.matmul(psw1[:], wr[(m1, 3)][:], o1r[:, bass.ts(s, 512)],
                                 start=True, stop=True)
                rw1 = big2.tile([128, 512], F32, tag="rw1")
                nc.scalar.activation(rw1[:], psw1[:], AF.Relu, bias=bcol(m1, 3))
                nc.vector.tensor_add(ht_n[:, bass.ts(s, 512)],
                                     o1r[:, bass.ts(s, 512)].bitcast(F32), rw1[:])
                nc.gpsimd.tensor_copy(htb_n[:, bass.ts(s, 512)],
                                      ht_n[:, bass.ts(s, 512)].bitcast(F32))
            ht, htb = ht_n, htb_n

        # ---------------- adaptive PMA (6 seed queries, shared across k) --
        if STAGE < 3:
            continue
        kb = work.tile([128, N], BF16, tag="kb")
        for s in range(4):
            psk = psb.tile([128, 512], F32, tag="ps")
            nc.tensor.matmul(psk[:], wr[(8, 1)][:], ht[:, bass.ts(s, 512)], start=True, stop=True)
            nc.scalar.activation(kb[:, bass.ts(s, 512)], psk[:], AF.Copy)
        vnb = work.tile([128, 16, 132], BF16, tag="vnb")
        nc.vector.memset(vnb[:, :, 128:129], 1.0)
        for g in range(4):
            psv = psb.tile([128, 512], F32, tag="ps")
            for cc in range(4):
                c = 4 * g + cc
                nc.tensor.matmul(psv[:, bass.ts(cc, 128)], htb[:, bass.ts(c, 128)],
                                 wvb[8][:], start=True, stop=True)
            nc.vector.tensor_copy(
                vnb[:, 4 * g:4 * g + 4, 0:128],
                psv[:].rearrange("p (a b) -> p a b", a=4))
        abt = work.tile([128, 16, 128], BF16, tag="abt")
        for c in range(16):
            pss = psb.tile([128, 512], F32, tag="ps")
            nc.tensor.matmul(pss[:, 0:128], kb[:, bass.ts(c, 128)], qmbd_b[:],
                             start=True, stop=True)
            nc.scalar.activation(abt[:, c, :], pss[:, 0:128], AF.Exp)
        psx = ps2.tile([128, 132], F32, tag="psx")
        for c in range(16):
            nc.tensor.matmul(psx[:, 0:129], abt[:, c, :], vnb[:, c, 0:129],
                             start=(c == 0), stop=(c == 15))
        rd = smal.tile([128, 1], F32, tag="rd")
        nc.vector.reciprocal_approx_fast(rd[:], psx[:, 128:129])
        onat = smal.tile([32, 128], F32, tag="onat")
        for h in range(4):
            sl = slice(h * 32, (h + 1) * 32)
            nc.vector.tensor_scalar_mul(onat[0:32, sl], psx[sl, sl], rd[sl, :])
        nc.vector.tensor_add(onat[:], onat[:], qmpb[:])
        pst = psb.tile([128, 512], F32, tag="ps")
        nc.tensor.transpose(pst[:, 0:32], onat[:], ident[0:32, 0:32])
        ot = smal.tile([128, 32], F32R, tag="ot")
        nc.vector.tensor_copy(ot[:], pst[:, 0:32])
        psw = psb.tile([128, 512], F32, tag="ps")
        nc.tensor.matmul(psw[:, 0:32], wr[(8, 3)][:], ot[:], start=True, stop=True)
        rw0 = smal.tile([128, 32], F32, tag="rw0")
        nc.vector.tensor_scalar(out=rw0[:], in0=psw[:, 0:32], scalar1=bcol(8, 3),
                                scalar2=0.0, op0=ALU.add, op1=ALU.max)
        zbt = smal.tile([128, 32], F32, tag="zbt")
        nc.vector.tensor_add(zbt[:], ot[:].bitcast(F32), rw0[:])
        # scatter prefix sets into the token pool
        for k in range(6):
            nc.vector.tensor_copy(tt[:, 21 * b + _tri(k): 21 * b + _tri(k) + k + 1],
                                  zbt[:, 0:k + 1])

    # ------------------ SAB x2 over the 168 stacked tokens ----------------
    if STAGE < 4:
        return
    cur = tt
    for s_i, m in enumerate((9, 10)):
        sbq = smal.tile([128, 1], F32, tag=f"sbq_{s_i}")
        nc.vector.tensor_scalar_mul(sbq[:], bia[:, m, 0:1], RSQ)
        psq = psb.tile([128, 512], F32, tag="ps")
        nc.tensor.matmul(psq[:, 0:NTOK], wr[(m, 0)][:], cur[:], start=True, stop=True)
        qs_f = work.tile([128, NTOK], F32, tag="qs_f")
        nc.vector.tensor_scalar_add(qs_f[:], psq[:, 0:NTOK], bcol(m, 0))
        qsb = work.tile([128, NTOK], BF16, tag="qsb")
        nc.vector.tensor_scalar(out=qsb[:], in0=psq[:, 0:NTOK], scalar1=RSQ,
                                scalar2=sbq[:, :], op0=ALU.mult, op1=ALU.add)
        psk = psb.tile([128, 512], F32, tag="ps")
        nc.tensor.matmul(psk[:, 0:NTOK], wr[(m, 1)][:], cur[:], start=True, stop=True)
        ksb = work.tile([128, NTOK], BF16, tag="ksb")
        nc.vector.tensor_copy(ksb[:], psk[:, 0:NTOK])
        psv = psb.tile([128, 512], F32, tag="ps")
        nc.tensor.matmul(psv[:, 0:NTOK], wr[(m, 2)][:], cur[:], start=True, stop=True)
        vst = work.tile([128, NTOK], F32, tag="vst")
        nc.vector.tensor_scalar_add(vst[:], psv[:, 0:NTOK], bcol(m, 2))
        # V natural (two partition tiles)
        pn1 = psb.tile([128, 512], F32, tag="ps")
        nc.tensor.transpose(pn1[:, 0:128], vst[:, 0:128], ident[:])
        vsn1 = work.tile([128, 128], BF16, tag="vsn1")
        nc.vector.tensor_copy(vsn1[:], pn1[:, 0:128])
        pn2 = psb.tile([128, 512], F32, tag="ps")
        nc.tensor.transpose(pn2[0:40, 0:128], vst[:, 128:NTOK], ident[:])
        vsn2 = work.tile([40, 128], BF16, tag="vsn2")
        nc.vector.tensor_copy(vsn2[:], pn2[0:40, 0:128])
        # Q natural (residual)
        qn1 = work.tile([128, 128], F32, tag="qn1")
        pq1 = psb.tile([128, 512], F32, tag="ps")
        nc.tensor.transpose(pq1[:, 0:128], qs_f[:, 0:128], ident[:])
        nc.vector.tensor_copy(qn1[:], pq1[:, 0:128])
        qn2 = work.tile([40, 128], F32, tag="qn2")
        pq2 = psb.tile([128, 512], F32, tag="ps")
        nc.tensor.transpose(pq2[0:40, 0:128], qs_f[:, 128:NTOK], ident[:])
        nc.vector.tensor_copy(qn2[:], pq2[0:40, 0:128])

        onr = []
        for (qt0, qlen, maskt, qn) in ((0, 128, mask1, qn1), (128, 40, mask2, qn2)):
            pso = ps2.tile([128, 132], F32, tag="psx")  # reused as [*,128] AV accum
            rdns = []
            emts = []
            for h in range(4):
                sl = slice(h * 32, (h + 1) * 32)
                pss = psb.tile([128, 512], F32, tag="ps")
                nc.tensor.matmul(pss[0:qlen, 0:NTOK], qsb[sl, qt0:qt0 + qlen],
                                 ksb[sl, :], start=True, stop=True,
                                 tile_position=(h * 32, 0))
                em = smal.tile([128, NTOK], BF16, tag="em")
                epss = smal.tile([128, NTOK], F32, tag="epss")
                nc.scalar.activation(epss[0:qlen, :], pss[0:qlen, 0:NTOK], AF.Exp)
                nc.vector.tensor_tensor(out=em[0:qlen, :], in0=epss[0:qlen, :],
                                        in1=maskt[0:qlen, :], op=ALU.mult)
                dn = smal.tile([128, 1], F32, tag="dn")
                nc.vector.tensor_reduce(out=dn[0:qlen, :], in_=em[0:qlen, :],
                                        axis=AX.X, op=ALU.add)
                rdn = smal.tile([128, 1], F32, tag=f"rdn_{h}")
                nc.vector.reciprocal_approx_fast(rdn[0:qlen, :], dn[0:qlen, :])
                rdns.append(rdn)
                # transpose Em -> [j, i] (two column blocks)
                e1p = psb.tile([128, 512], BF16, tag="ps")
                nc.tensor.transpose(e1p[:, 0:qlen], em[0:qlen, 0:128], identb[0:qlen, 0:qlen])
                e1 = smal.tile([128, 128], BF16, tag="e1")
                nc.vector.tensor_copy(e1[:, 0:qlen], e1p[:, 0:qlen])
                e2p = psb.tile([128, 512], BF16, tag="ps")
                nc.tensor.transpose(e2p[0:40, 0:qlen], em[0:qlen, 128:NTOK], identb[0:qlen, 0:qlen])
                e2 = smal.tile([40, 128], BF16, tag="e2")
                nc.vector.tensor_copy(e2[:, 0:qlen], e2p[0:40, 0:qlen])
                emts.append((e1, e2))
            for h in range(4):
                sl = slice(h * 32, (h + 1) * 32)
                e1, e2 = emts[h]
                nc.tensor.matmul(pso[0:qlen, sl], e1[:, 0:qlen], vsn1[:, sl],
                                 start=True, stop=False)
                nc.tensor.matmul(pso[0:qlen, sl], e2[:, 0:qlen], vsn2[:, sl],
                                 start=False, stop=True)
            on = work.tile([128, 128], F32, tag="on")
            for h in range(4):
                sl = slice(h * 32, (h + 1) * 32)
                nc.vector.tensor_scalar_mul(on[0:qlen, sl], pso[0:qlen, sl],
                                            rdns[h][0:qlen, :])
            nc.vector.tensor_add(on[0:qlen, :], on[0:qlen, :], qn[0:qlen, :])
            onr.append((on, qlen))
        # back to transposed layout + fc_o
        ots = work.tile([128, NTOK], F32R, tag="ots")
        po1 = psb.tile([128, 512], F32, tag="ps")
        nc.tensor.transpose(po1[:, 0:128], onr[0][0][0:128, :], ident[:])
        nc.vector.tensor_copy(ots[:, 0:128], po1[:, 0:128])
        po2 = psb.tile([128, 512], F32, tag="ps")
        nc.tensor.transpose(po2[:, 0:40], onr[1][0][0:40, :], ident[0:40, 0:40])
        nc.vector.tensor_copy(ots[:, 128:NTOK], po2[:, 0:40])
        psw = psb.tile([128, 512], F32, tag="ps")
        nc.tensor.matmul(psw[:, 0:NTOK], wr[(m, 3)][:], ots[:], start=True, stop=True)
        rw = smal.tile([128, NTOK], F32, tag="rw")
        nc.scalar.activation(rw[:], psw[:, 0:NTOK], AF.Relu, bias=bcol(m, 3))
        nxt = toks.tile([128, NTOK], F32R, tag=f"tt{s_i + 1}")
        nc.vector.tensor_add(nxt[:], ots[:].bitcast(F32), rw[:])
        cur = nxt

    # ------------------ fc head, v-means, selection -----------------------
    if STAGE < 5:
        return
    psf = psb.tile([128, 512], F32, tag="ps")
    nc.tensor.matmul(psf[0:6, 0:NTOK], fcw[:, 0:6], cur[:], start=True, stop=True)
    fct = work.tile([6, NTOK], F32, tag="fct")
    nc.vector.tensor_scalar_add(fct[:], psf[0:6, 0:NTOK], fcb[0:6, :])
    pf1 = psb.tile([128, 512], F32, tag="ps")
    nc.tensor.transpose(pf1[:, 0:6], fct[:, 0:128], ident[0:6, 0:6])
    fn1 = smal.tile([128, 6], F32R, tag="fn1")
    nc.vector.tensor_copy(fn1[:], pf1[:, 0:6])
    pf2 = psb.tile([128, 512], F32, tag="ps")
    nc.tensor.transpose(pf2[0:40, 0:6], fct[:, 128:NTOK], ident[0:6, 0:6])
    fn2 = smal.tile([40, 6], F32R, tag="fn2")
    nc.vector.tensor_copy(fn2[:], pf2[0:40, 0:6])
    psvs = psb.tile([128, 512], F32, tag="ps")
    nc.tensor.matmul(psvs[0:112, 0:6], avgsel1[:], fn1[:], start=True, stop=False)
    nc.tensor.matmul(psvs[0:112, 0:6], avgsel2[:], fn2[:], start=False, stop=True)
    sel = work.tile([48, 6], F32, tag="sel")
    nc.vector.tensor_copy(sel[0:32, :], psvs[64:96, 0:6])
    nc.vector.tensor_copy(sel[32:48, :], psvs[96:112, 0:6])
    vl48 = smal.tile([48, 1], F32, tag="vl48")
    nc.vector.tensor_copy(vl48[:], psvs[0:48, 0:1])

    # ------------------ params_out ---------------------------------------
    pout = work.tile([48, 4], F32, tag="pout")
    nc.vector.tensor_copy(pout[:, 0:2], sel[:, 2:4])
    nc.scalar.activation(pout[:, 2:4], sel[:, 4:6], AF.Exp)
    nc.scalar.activation(pout[:, 2:4], pout[:, 2:4], AF.Ln, bias=1.0)
    nc.vector.tensor_scalar_mul(pout[:], pout[:], valid48[:, :])
    nc.vector.tensor_scalar_add(pout[:], pout[:], inv48[:, :])
    nc.sync.dma_start(out=ext["pout_e"][:], in_=pout[:])

    if STAGE < 6:
        return
    # ------------------ mixture weights (pi) ------------------------------
    pst = psb.tile([128, 512], F32, tag="ps")
    nc.tensor.transpose(pst[0:1, 0:48], sel[:, 1:2], ident[0:48, 0:48])
    pl = smal.tile([1, 48], F32, tag="pl")
    nc.vector.tensor_tensor(out=pl[:], in0=pst[0:1, 0:48], in1=validf[:], op=ALU.mult)
    nc.vector.tensor_add(pl[:], pl[:], neginvf[:])
    epl = smal.tile([1, 48], F32, tag="epl")
    nc.scalar.activation(epl[:], pl[:], AF.Exp)
    den8 = smal.tile([1, 8], F32, tag="den8")
    nc.vector.tensor_reduce(out=den8[:], in_=epl[:].rearrange("p (a b) -> p a b", a=8),
                            axis=AX.X, op=ALU.add)
    ld8 = smal.tile([1, 8], F32, tag="ld8")
    nc.scalar.activation(ld8[:], den8[:], AF.Ln)
    ldb = smal.tile([1, 48], F32, tag="ldb")
    ld8_ap = ld8[:]
    nc.vector.tensor_copy(ldb[:].rearrange("p (a b) -> p a b", a=8),
                          bass.AP(tensor=ld8_ap.tensor, offset=ld8_ap.offset,
                                  ap=[ld8_ap.ap[0], [1, 8], [0, 6]]))
    lpi = smal.tile([1, 48], F32, tag="lpi")
    nc.vector.tensor_sub(lpi[:], pl[:], ldb[:])
    nc.vector.tensor_tensor(out=lpi[:], in0=lpi[:], in1=validf[:], op=ALU.mult)
    nc.vector.tensor_add(lpi[:], lpi[:], neg23f[:])

    # sigma stats for the log-prob stage
    lg = smal.tile([48, 2], F32, tag="lg")
    nc.scalar.activation(lg[:], pout[:, 2:4], AF.Ln)
    lgs = smal.tile([48, 1], F32, tag="lgs")
    nc.vector.tensor_reduce(out=lgs[:], in_=lg[:], axis=AX.X, op=ALU.add)
    rs = smal.tile([48, 2], F32, tag="rs")
    nc.vector.reciprocal_approx_fast(rs[:], pout[:, 2:4])

    # stage to DRAM: pk[48, 8] = mu0 mu1 rs0 rs1 lgs lpi
    dram = stk.enter_context(tc.tile_pool(name="dram", bufs=1, space="DRAM"))
    pk = dram.tile([48, 8], F32)
    nc.sync.dma_start(out=pk[:, 0:2], in_=pout[:, 0:2])
    nc.sync.dma_start(out=pk[:, 2:4], in_=rs[:])
    nc.sync.dma_start(out=pk[:, 4:5], in_=lgs[:])
    pk_ap = pk[:]
    nc.sync.dma_start(out=bass.AP(tensor=pk_ap.tensor, offset=pk_ap.offset + 5, ap=[[8, 48]]),
                      in_=lpi[:])
    vstg = dram.tile([48, 1], F32)
    nc.sync.dma_start(out=vstg[:], in_=vl48[:])

    # ------------------ log-likelihood over all (b, n) --------------------
    def pk_bcast(col):
        t = smal.tile([128, 6], F32, tag=f"pkb_{col}")
        pk_ap = pk[:]
        for k in range(6):
            src = bass.AP(tensor=pk_ap.tensor, offset=pk_ap.offset + k * 8 + col,
                          ap=[[48, 8], [0, 16]])
            nc.sync.dma_start(out=t[:, k:k + 1], in_=src)
        return t

    if STAGE < 7:
        return
    mu0b, mu1b = pk_bcast(0), pk_bcast(1)
    rs0b, rs1b = pk_bcast(2), pk_bcast(3)
    lgsb, lpib = pk_bcast(4), pk_bcast(5)
    ec = smal.tile([128, 6], F32, tag="ec")
    nc.vector.tensor_sub(ec[:], lpib[:], lgsb[:])
    nc.scalar.activation(ec[:], ec[:], AF.Exp, bias=-LOG2PI)
    xs0 = work.tile([128, 128], F32, tag="xs0")
    nc.sync.dma_start(out=xs0[:], in_=ext["xlp0"][:])
    xs1 = work.tile([128, 128], F32, tag="xs1")
    nc.sync.dma_start(out=xs1[:], in_=ext["xlp1"][:])

    def bc6(t):  # [128, 6] -> broadcast over nl (inner layout nl,k)
        a = t[:]
        return bass.AP(tensor=a.tensor, offset=a.offset, ap=[a.ap[0], [0, 128], [1, 6]])

    def bcx(t):  # [128, 128] x -> repeat over k
        a = t[:]
        return bass.AP(tensor=a.tensor, offset=a.offset, ap=[a.ap[0], [1, 128], [0, 6]])

    acc = lpp.tile([128, 128, 6], F32, tag="acc")
    z1t = lpp.tile([128, 128, 6], F32, tag="z1t")
    nc.vector.tensor_tensor(out=acc[:], in0=bcx(xs0), in1=bc6(mu0b), op=ALU.subtract)
    nc.vector.tensor_tensor(out=acc[:], in0=acc[:], in1=bc6(rs0b), op=ALU.mult)
    nc.scalar.activation(acc[:], acc[:], AF.Square)
    nc.vector.tensor_tensor(out=z1t[:], in0=bcx(xs1), in1=bc6(mu1b), op=ALU.subtract)
    nc.vector.tensor_tensor(out=z1t[:], in0=z1t[:], in1=bc6(rs1b), op=ALU.mult)
    nc.scalar.activation(z1t[:], z1t[:], AF.Square)
    nc.vector.tensor_add(acc[:], acc[:], z1t[:])
    nc.scalar.activation(acc[:], acc[:], AF.Exp, scale=-0.5)
    nc.vector.tensor_tensor(out=acc[:], in0=acc[:], in1=bc6(ec), op=ALU.mult)
    ssum = work.tile([128, 128], F32, tag="ssum")
    nc.vector.tensor_reduce(out=ssum[:], in_=acc[:], axis=AX.X, op=ALU.add)
    nc.scalar.activation(ssum[:], ssum[:], AF.Ln)
    llp = smal.tile([128, 1], F32, tag="llp")
    nc.vector.tensor_reduce(out=llp[:], in_=ssum[:], axis=AX.X, op=ALU.add)
    nc.sync.dma_start(out=ext["llp_e"][:], in_=llp[:])

    if STAGE < 8:
        return
    # ------------------ BCE on cumulative halting logits -------------------
    vl8 = smal.tile([8, 6], F32, tag="vl8")
    vstg_ap = vstg[:]
    nc.sync.dma_start(out=vl8[:], in_=bass.AP(tensor=vstg_ap.tensor, offset=vstg_ap.offset,
                                              ap=[[6, 8], [1, 6]]))
    sg = smal.tile([8, 6], F32, tag="sg")
    nc.scalar.activation(sg[:], vl8[:], AF.Sigmoid)
    lv = smal.tile([8, 6], F32, tag="lv")
    nc.scalar.activation(lv[:], sg[:], AF.Ln, bias=1e-10)
    lc = smal.tile([8, 6], F32, tag="lc")
    nc.vector.tensor_copy(lc[:], lv[:])
    for j in range(1, 6):
        nc.vector.tensor_add(lc[:, j:j + 1], lc[:, j - 1:j], lv[:, j:j + 1])
    exl = smal.tile([8, 6], F32, tag="exl")
    nc.scalar.activation(exl[:], lc[:], AF.Exp)
    t1 = smal.tile([8, 6], F32, tag="t1")
    nc.vector.tensor_scalar(out=t1[:], in0=exl[:], scalar1=-1.0, scalar2=1.0 + 1e-10,
                            op0=ALU.mult, op1=ALU.add)
    nc.scalar.activation(t1[:], t1[:], AF.Ln)
    cl = smal.tile([8, 6], F32, tag="cl")
    nc.vector.tensor_sub(cl[:], lc[:], t1[:])
    mx = smal.tile([8, 6], F32, tag="mx")
    nc.vector.tensor_scalar_max(mx[:], cl[:], 0.0)
    ct = smal.tile([8, 6], F32, tag="ct")
    nc.vector.tensor_tensor(out=ct[:], in0=cl[:], in1=ctarg[:], op=ALU.mult)
    nc.vector.tensor_sub(mx[:], mx[:], ct[:])
    ab = smal.tile([8, 6], F32, tag="ab")
    nc.scalar.activation(ab[:], cl[:], AF.Abs)
    nc.scalar.activation(ab[:], ab[:], AF.Exp, scale=-1.0)
    nc.scalar.activation(ab[:], ab[:], AF.Ln, bias=1.0)
    bcel = smal.tile([8, 6], F32, tag="bcel")
    nc.vector.tensor_add(bcel[:], mx[:], ab[:])
    nc.sync.dma_start(out=ext["bcel_e"][:], in_=bcel[:])


# ---------------------------------------------------------------------------
# host side
# ---------------------------------------------------------------------------

def _np(x):
    return np.asarray(x, dtype=np.float32)


def _prep_inputs(X, params, K_true):
    """Build per-core input maps."""
    wmat = np.zeros((11, 4, 128, 128), np.float32)
    wbias = np.zeros((128, 11, 4), np.float32)
    indT = np.zeros((128, 4, 32), np.float32)
    mabs = []
    for l in range(4):
        mabs.append(params["isab"][l]["mab0"])
        mabs.append(params["isab"][l]["mab1"])
    mabs.append(params["apma"]["mab"])
    mabs.append(params["sab"][0])
    mabs.append(params["sab"][1])
    for m, p in enumerate(mabs):
        for j, key in enumerate(("q", "k", "v", "o")):
            w = _np(p[key]["w"])
            wmat[m, j, :w.shape[0], :] = w
            wbias[:, m, j] = _np(p[key]["b"])
    for l in range(4):
        indT[:, l, :] = _np(params["isab"][l]["I"])[0].T

    # seed recurrence (parameter-only preprocessing)
    sfw = _np(params["apma"]["seed_fc"]["w"])
    sfb = _np(params["apma"]["seed_fc"]["b"])
    s = _np(params["apma"]["seed"])
    seeds = np.zeros((6, 128), np.float32)
    for k in range(6):
        seeds[k] = s
        s = np.tanh(s @ sfw + sfb)
    qm = seeds @ _np(params["apma"]["mab"]["q"]["w"]) + _np(params["apma"]["mab"]["q"]["b"])
    qmbd = np.zeros((128, 128), np.float32)
    for h in range(4):
        qmbd[h * 32:(h + 1) * 32, h * 32:h * 32 + 6] = qm.T[h * 32:(h + 1) * 32] * RSQ
    qmpb = np.zeros((32, 128), np.float32)
    qmpb[0:6] = qm + _np(params["apma"]["mab"]["v"]["b"])[None, :]

    hsel = np.zeros((128, 128), np.float32)
    for h in range(4):
        hsel[h * 32:(h + 1) * 32, h * 32:(h + 1) * 32] = 1.0
    ident = np.eye(128, dtype=np.float32)

    # token bookkeeping: token t = b*21 + tri(k) + i
    sabmask = np.zeros((NTOK, NTOK), np.float32)
    setid = np.empty(NTOK, np.int64)
    for b in range(BSH):
        for k in range(6):
            t0 = 21 * b + _tri(k)
            setid[t0:t0 + k + 1] = b * 6 + k
    sabmask[setid[:, None] == setid[None, :]] = 1.0

    avg = np.zeros((NTOK, 112), np.float32)
    for b in range(BSH):
        for k in range(6):
            t0 = 21 * b + _tri(k)
            avg[t0:t0 + k + 1, b * 6 + k] = 1.0 / (k + 1)

    fcw = np.zeros((128, 8), np.float32)
    fcw[:, 0:6] = _np(params["fc"]["w"])
    fcb = np.zeros((8, 1), np.float32)
    fcb[0:6, 0] = _np(params["fc"]["b"])

    X = _np(X)
    K_true = np.asarray(K_true).astype(np.int64)
    in_maps = []
    for c in range(NCORES):
        Xc = X[c * BSH:(c + 1) * BSH]           # [8, 2048, 2]
        kt = K_true[c * BSH:(c + 1) * BSH]      # [8]
        avgsel = avg.copy()
        for b in range(BSH):
            kb = int(kt[b])
            t0 = 21 * b + _tri(kb - 1)
            for j in range(kb):
                avgsel[t0 + j, 64 + b * 6 + j] = 1.0
        valid = (np.arange(6)[None, :] < kt[:, None]).astype(np.float32)  # [8, 6]
        v48 = valid.reshape(48, 1)
        ctarg = (np.arange(6)[None, :] < (kt - 1)[:, None]).astype(np.float32)
        in_maps.append({
            "x_t": np.ascontiguousarray(Xc.transpose(0, 2, 1)),
            "xlp0": np.ascontiguousarray(Xc[:, :, 0].reshape(128, 128)),
            "xlp1": np.ascontiguousarray(Xc[:, :, 1].reshape(128, 128)),
            "wmat": wmat, "wbias": wbias, "indT": indT,
            "qmbd": qmbd, "qmpb": qmpb, "hsel": hsel, "ident": ident,
            "sabmask": sabmask, "avgsel": avgsel,
            "valid48": v48, "inv48": 1.0 - v48,
            "validf": v48.reshape(1, 48).copy(),
            "neginvf": ((v48 - 1.0) * 1e10).reshape(1, 48).copy(),
            "neg23f": ((1.0 - v48) * LN1EM10).reshape(1, 48).copy(),
            "ctarg": ctarg, "fcw": fcw, "fcb": fcb,
        })
    return in_maps


def kernel(X, params, K_true, K_max):
    assert int(K_max) == KM
    if "nc" not in _NC_CACHE:
        _NC_CACHE["nc"] = _build_nc()
    nc = _NC_CACHE["nc"]
    in_maps = _prep_inputs(X, params, K_true)
    res = run_bass_kernel_spmd(nc, in_maps, core_ids=list(range(NCORES)))
    pouts, lls, bcs = [], [], []
    for c in range(NCORES):
        r = res.results[c]
        pouts.append(r["pout"].reshape(BSH, 6, 4))
        lls.append(r["llp"].sum())
        bcs.append(r["bcel"].mean())
    params_out = np.concatenate(pouts, 0).astype(np.float32)
    ll = np.float32(np.sum(lls) / (B * N))
    bc = np.float32(np.mean(bcs))
    return params_out, ll, bc
